# revision 35
# baseline (speedup 1.0000x reference)
"""L1HyMixDe denoiser on 8 Trainium2 NeuronCores.

Pipeline: adaptive median + 191x191 noise-whitening/eigendecomposition on host
(tiny LAPACK ops), then the full 40-iteration ADMM (eigen-projection, per-image
2D-DCT soft-threshold denoise, soft-threshold state updates) and the final
reconstruction run on-device, sharded over the spatial axis (2048 pixels/core).
Per iteration the K=10 eigen-images are re-assembled with an AllGather.
Matmul operands and ADMM state are bf16 (PSUM accumulation fp32).
"""
import numpy as np

ROW, COL, BAND = 128, 128, 191
K = 10
MAX_WIN = 7
ITERS = 40
NCORES = 8
NLOC = (ROW * COL) // NCORES          # 2048 pixels per core
RLOC = ROW // NCORES                  # 16 image rows per core
KB = K * COL                          # 1280: batched DCT width
B0, B1 = 128, BAND - 128              # band chunks: 128 + 63
N = ROW * COL
TAU = float(np.sqrt(2.0 * np.log(float(N))))

_CACHED = {}


# ----------------------------------------------------------------- host side

def _dct_mat(n, dtype=np.float32):
    j = np.arange(n)
    k = np.arange(n)[:, None]
    C = np.cos(np.pi * (2 * j[None, :] + 1) * k / (2 * n))
    C *= np.sqrt(2.0 / n)
    C[0] *= np.sqrt(0.5)
    return np.ascontiguousarray(C.astype(dtype))


def _adaptive_median(img):
    """Adaptive median (windows 3,5,7): rad-1 everywhere via np.partition,
    rad-2/3 only at pixels where rad-1 is invalid (ties, mostly edge pad)."""
    H, W, B = img.shape
    r = MAX_WIN // 2
    xp = np.pad(img, ((r, r), (r, r), (0, 0)), mode="edge")
    offs = [(dy - r, dx - r) for dy in range(MAX_WIN) for dx in range(MAX_WIN)]

    def stack(rad, mask=None):
        sel = [i for i, (dy, dx) in enumerate(offs)
               if max(abs(dy), abs(dx)) <= rad]
        views = []
        for i in sel:
            dy, dx = offs[i]
            v = xp[r + dy:r + dy + H, r + dx:r + dx + W]
            views.append(v[mask] if mask is not None else v)
        return np.stack(views, axis=0)

    st = stack(1)
    m = st.shape[0]
    part = np.partition(st, [0, m // 2, m - 1], axis=0)
    zmin, zmed, zmax = part[0], part[m // 2], part[m - 1]
    valid = (zmin < zmed) & (zmed < zmax)
    out = np.where(valid, np.where((zmin < img) & (img < zmax), img, zmed), img)
    done = valid.copy()
    zmed_last = zmed

    bad = ~done
    if bad.any():
        for rad in (2, 3):
            stb = stack(rad, mask=bad)
            m = stb.shape[0]
            part = np.partition(stb, [0, m // 2, m - 1], axis=0)
            zminb, zmedb, zmaxb = part[0], part[m // 2], part[m - 1]
            validb = (zminb < zmedb) & (zmedb < zmaxb)
            imgb = img[bad]
            stageb = np.where((zminb < imgb) & (imgb < zmaxb), imgb, zmedb)
            ob = out[bad]
            out[bad] = np.where(validb & ~done[bad], stageb, ob)
            zl = zmed_last[bad]
            zl[:] = zmedb
            zmed_last[bad] = zl
            done[bad] = done[bad] | validb
            bad = ~done
            if not bad.any():
                break
    return np.where(done, out, zmed_last)


def _host_prep(img, p):
    dtype = np.float32
    img = np.asarray(img, dtype)
    y_og = img.reshape(N, BAND).T
    img_median = _adaptive_median(img)
    img_ro = np.where(np.abs(img - img_median) > p, img_median, img)
    y_ro = img_ro.reshape(N, BAND).T

    eps = dtype(1e-6)
    RR = (y_ro @ y_ro.T).astype(dtype)
    RRi = np.linalg.inv(RR + eps * np.eye(BAND, dtype=dtype)).astype(dtype)
    di = np.diag(RRi)
    M_ = RRi @ RR @ RRi                       # rw_diag without a second data pass
    rw_diag = (np.diag(M_) / (di * di) / N).astype(dtype)

    s = (1.0 / np.sqrt(rw_diag)).astype(dtype)
    y_w = (y_og * s[:, None]).astype(dtype)
    C = (s[:, None] * RR * s[None, :] / N).astype(dtype)
    _, evecs = np.linalg.eigh(C)
    e = np.ascontiguousarray(evecs[:, ::-1][:, :K]).astype(dtype)

    v0 = img_median.reshape(N, BAND).T.astype(dtype)
    s0 = (y_w - v0).astype(dtype)             # s_0 = y - v0 + d0, d0 = 0
    return y_w, s0, e, s


# --------------------------------------------------------------- device side

def _build_kernel(iters):
    import concourse.bass as bass
    import concourse.mybir as mybir
    import concourse.tile as tile
    from concourse import bacc

    f32 = mybir.dt.float32
    bf = mybir.dt.bfloat16
    nc = bacc.Bacc("TRN2", target_bir_lowering=False, debug=False,
                   num_devices=NCORES)

    yw0_d = nc.declare_dram_parameter("yw0", [B0, NLOC], bf, isOutput=False)
    yw1_d = nc.declare_dram_parameter("yw1", [B1, NLOC], bf, isOutput=False)
    s00_d = nc.declare_dram_parameter("s00", [B0, NLOC], bf, isOutput=False)
    s01_d = nc.declare_dram_parameter("s01", [B1, NLOC], bf, isOutput=False)
    e_d = nc.declare_dram_parameter("e", [BAND, K], bf, isOutput=False)
    e2_d = nc.declare_dram_parameter("e2", [BAND, K], bf, isOutput=False)
    eT_d = nc.declare_dram_parameter("eT", [K, BAND], bf, isOutput=False)
    dct_d = nc.declare_dram_parameter("dct", [ROW, ROW], bf, isOutput=False)
    dctT_d = nc.declare_dram_parameter("dctT", [ROW, ROW], bf, isOutput=False)
    dloc_d = nc.declare_dram_parameter("dloc", [ROW, RLOC], bf, isOutput=False)
    wT_d = nc.declare_dram_parameter("wT", [K, BAND], bf, isOutput=False)
    out_d = nc.declare_dram_parameter("out", [BAND, NLOC], f32, isOutput=True)

    NQ = 4              # quarters of NLOC
    Q = NLOC // NQ      # 512

    with tile.TileContext(nc) as tc:
        with (
            tc.tile_pool(name="state", bufs=1) as state,
            tc.tile_pool(name="consts", bufs=1) as consts,
            tc.tile_pool(name="work", bufs=2) as work,
            tc.tile_pool(name="tmp", bufs=3) as tmp,
            tc.tile_pool(name="ps_big", bufs=2, space="PSUM") as ps_big,
            tc.tile_pool(name="ps_dct", bufs=2, space="PSUM") as ps_dct,
            tc.tile_pool(name="dram", bufs=2, space="DRAM") as dram,
        ):
            # ---- persistent state & constants in SBUF (bf16)
            yw0 = state.tile([B0, NLOC], bf, tag="yw0")
            yw1 = state.tile([B1, NLOC], bf, tag="yw1")
            st0 = state.tile([B0, NLOC], bf, tag="st0")
            st1 = state.tile([B1, NLOC], bf, tag="st1")
            dd0 = state.tile([B0, NLOC], bf, tag="dd0")
            dd1 = state.tile([B1, NLOC], bf, tag="dd1")
            t1_0 = state.tile([B0, NLOC], bf, tag="t1_0")
            t1_1 = state.tile([B1, NLOC], bf, tag="t1_1")
            zsb = state.tile([K, NLOC], bf, tag="zsb")
            ybig_a = state.tile([ROW, KB // 2], bf, tag="ybig_a")
            ybig_b = state.tile([ROW, KB // 2], bf, tag="ybig_b")
            eysb = state.tile([K, NLOC], bf, tag="eysb")
            ez0s = state.tile([B0, NLOC], bf, tag="ez0s")
            ez1s = state.tile([B1, NLOC], bf, tag="ez1s")

            e0 = consts.tile([B0, K], bf, tag="e0")
            e1 = consts.tile([B1, K], bf, tag="e1")
            e20 = consts.tile([B0, K], bf, tag="e20")
            e21 = consts.tile([B1, K], bf, tag="e21")
            eT = consts.tile([K, BAND], bf, tag="eT")
            dsb = consts.tile([ROW, ROW], bf, tag="dsb")
            dTsb = consts.tile([ROW, ROW], bf, tag="dTsb")
            dloc = consts.tile([ROW, RLOC], bf, tag="dloc")
            wT = consts.tile([K, BAND], bf, tag="wT")



            nc.sync.dma_start(out=yw0[:], in_=yw0_d[:])
            nc.sync.dma_start(out=yw1[:], in_=yw1_d[:])
            nc.sync.dma_start(out=st0[:], in_=s00_d[:])
            nc.sync.dma_start(out=st1[:], in_=s01_d[:])
            nc.sync.dma_start(out=e0[:], in_=e_d[0:B0, :])
            nc.sync.dma_start(out=e1[:], in_=e_d[B0:BAND, :])
            nc.sync.dma_start(out=e20[:], in_=e2_d[0:B0, :])
            nc.sync.dma_start(out=e21[:], in_=e2_d[B0:BAND, :])
            nc.sync.dma_start(out=eT[:], in_=eT_d[:])
            nc.sync.dma_start(out=dsb[:], in_=dct_d[:])
            nc.sync.dma_start(out=dTsb[:], in_=dctT_d[:])
            nc.sync.dma_start(out=dloc[:], in_=dloc_d[:])
            nc.sync.dma_start(out=wT[:], in_=wT_d[:])
            nc.vector.memset(dd0[:], 0.0)
            nc.vector.memset(dd1[:], 0.0)

            HL = NLOC // 2      # 1024: half width (bf16 moving max)
            dmae = [nc.sync, nc.gpsimd, nc.sync, nc.scalar]

            for t in range(iters):
                # ---- eigen_y = e.T @ s = e.T @ t1 + (2e).T @ d  (K x NLOC)
                ag_in = dram.tile([K, NLOC], bf, tag="ag_in")
                for h in range(2):
                    hs = slice(h * HL, (h + 1) * HL)
                    # two quarter-chains packed in disjoint 32-column groups
                    # of the PE array (out M=10 << 32) -> run concurrently
                    ey_ps = ps_big.tile([64, HL], f32, tag="psq")
                    if t == 0:
                        steps = ((e0, e1), (st0, st1))
                        chain = [(e0, st0), (e1, st1)]
                    else:
                        chain = [(e20, dd0), (e21, dd1),
                                 (e0, t1_0), (e1, t1_1)]
                    nstep = len(chain)
                    for step, (lhs, rhs) in enumerate(chain):
                        for j in range(2):
                            js = slice(h * HL + j * Q, h * HL + (j + 1) * Q)
                            ps = ey_ps[32 * j:32 * j + K, j * Q:(j + 1) * Q]
                            nc.tensor.matmul(ps, lhs[:], rhs[:, js],
                                             start=(step == 0),
                                             stop=(step == nstep - 1),
                                             tile_position=(0, 32 * j))
                    for j in range(2):
                        qs = slice(h * HL + j * Q, h * HL + (j + 1) * Q)
                        pj = ey_ps[32 * j:32 * j + K, j * Q:(j + 1) * Q]
                        if j == 0:
                            nc.scalar.copy(eysb[:, qs], pj)
                        else:
                            nc.vector.tensor_copy(eysb[:, qs], pj)
                    dmae[h].dma_start(out=ag_in[:, hs], in_=eysb[:, hs])

                ag_out = dram.tile([NCORES, K, NLOC], bf, tag="ag_out")
                nc.gpsimd.collective_compute(
                    "AllGather",
                    mybir.AluOpType.bypass,
                    replica_groups=[list(range(NCORES))],
                    ins=[ag_in.opt()],
                    outs=[ag_out.opt()],
                )
                # gather into (row, k*COL + col) layout, group 0 first so its
                # DCT can begin while group 1's DMAs drain
                for g, ybig in ((0, ybig_a), (1, ybig_b)):
                    for rk in range(NCORES):
                        src = ag_out[rk, g * (K // 2):(g + 1) * (K // 2), :]                             .rearrange("k (r w) -> r k w", r=RLOC)
                        dmae[rk % 4].dma_start(
                            out=ybig[rk * RLOC:(rk + 1) * RLOC, :]
                            .rearrange("r (k w) -> r k w", k=K // 2),
                            in_=src,
                        )


                # ---- DCT denoise: two k-groups of 5, pipelined across
                # engines (PE / DVE / ACT stages of group 0 overlap group 1)
                KB2 = KB // 2          # 640
                KH = K // 2            # 5
                for g in range(2):
                    ybig = ybig_a if g == 0 else ybig_b
                    qt_g = ps_dct.tile([ROW, KB2], f32, tag="dctps")
                    for k in range(KH):
                        nc.tensor.matmul(
                            qt_g[:, k * COL:(k + 1) * COL],
                            ybig[:, k * COL:(k + 1) * COL], dTsb[:],
                            start=True, stop=True)
                    qt_sb = work.tile([ROW, KB2], bf, tag="qt_sb")
                    nc.vector.tensor_copy(qt_sb[:], qt_g[:])

                    ct_g = ps_dct.tile([ROW, KB2], f32, tag="dctps")
                    nc.tensor.matmul(ct_g[:, 0:512], dTsb[:], qt_sb[:, 0:512],
                                     start=True, stop=True)
                    nc.tensor.matmul(ct_g[:, 512:KB2], dTsb[:],
                                     qt_sb[:, 512:KB2],
                                     start=True, stop=True)
                    ct_sb = tmp.tile([ROW, KB2], bf, tag="ct_sb")
                    nc.scalar.copy(ct_sb[:], ct_g[:])
                    cl_sb = tmp.tile([ROW, KB2], bf, tag="cl_sb")
                    nc.vector.tensor_scalar_max(cl_sb[:], ct_sb[:], -TAU)
                    cl2_sb = tmp.tile([ROW, KB2], bf, tag="cl2_sb")
                    nc.vector.tensor_scalar_min(cl2_sb[:], cl_sb[:], TAU)
                    cpt_sb = work.tile([ROW, KB2], bf, tag="cpt_sb")
                    nc.vector.tensor_sub(cpt_sb[:], ct_sb[:], cl2_sb[:])

                    n1_g = ps_dct.tile([ROW, KB2], f32, tag="dctps")
                    for k in range(KH):
                        nc.tensor.matmul(
                            n1_g[:, k * COL:(k + 1) * COL],
                            cpt_sb[:, k * COL:(k + 1) * COL], dsb[:],
                            start=True, stop=True)
                    n1_sb = work.tile([ROW, KB2], bf, tag="n1_sb")
                    nc.vector.tensor_copy(n1_sb[:], n1_g[:])

                    zl_g = ps_dct.tile([RLOC, KB2], f32, tag="dctps")
                    nc.tensor.matmul(zl_g[:, 0:512], dloc[:], n1_sb[:, 0:512],
                                     start=True, stop=True)
                    nc.tensor.matmul(zl_g[:, 512:KB2], dloc[:],
                                     n1_sb[:, 512:KB2],
                                     start=True, stop=True)
                    zs_sb = tmp.tile([RLOC, KB2], bf, tag="zs_sb")
                    for k in range(KH):
                        ks = slice(k * COL, (k + 1) * COL)
                        kg = g * KH + k
                        if k % 2 == 0:
                            nc.scalar.copy(zs_sb[:, ks], zl_g[:, ks])
                        else:
                            nc.vector.tensor_copy(zs_sb[:, ks], zl_g[:, ks])
                        dmae[kg % 4].dma_start(
                            out=zsb[kg:kg + 1, :]
                            .rearrange("a (r w) -> a r w", r=RLOC),
                            in_=zs_sb[:, ks],
                        )


                if t == iters - 1:
                    break

                # ---- EZ = e @ z ; state updates pipelined in halves
                # t1 = EZ - d; a = y - t1; d' = clip(a)
                for h in range(2):
                    hs = slice(h * HL, (h + 1) * HL)
                    ez0 = ps_big.tile([B0, HL], f32, tag="psq")
                    ez1 = ps_big.tile([B1, HL], f32, tag="psq")
                    for j in range(2):
                        js = slice(h * HL + j * Q, h * HL + (j + 1) * Q)
                        nc.tensor.matmul(ez0[:, j * Q:(j + 1) * Q],
                                         eT[:, 0:B0], zsb[:, js],
                                         start=True, stop=True)
                        nc.tensor.matmul(ez1[:, j * Q:(j + 1) * Q],
                                         eT[:, B0:BAND], zsb[:, js],
                                         start=True, stop=True)
                    nc.scalar.copy(ez0s[:, hs], ez0[:])
                    nc.scalar.copy(ez1s[:, hs], ez1[:])
                    for (ezs, ddt, yyt, t1t) in (
                        (ez0s, dd0, yw0, t1_0),
                        (ez1s, dd1, yw1, t1_1),
                    ):
                        P = ddt.shape[0]
                        a_ = tmp.tile([P, HL], bf, tag="a_")
                        b_ = tmp.tile([P, HL], bf, tag="b_")
                        nc.vector.tensor_sub(t1t[:, hs], ezs[:, hs], ddt[:, hs])
                        nc.vector.tensor_sub(a_[:], yyt[:, hs], t1t[:, hs])
                        nc.vector.tensor_scalar_max(b_[:], a_[:], -1.0)
                        nc.vector.tensor_scalar_min(ddt[:, hs], b_[:], 1.0)


            # ---- reconstruction: out = W @ z
            osb0 = state.tile([B0, NLOC], f32, tag="osb0")
            osb1 = state.tile([B1, NLOC], f32, tag="osb1")
            for h in range(2):
                hs = slice(h * HL, (h + 1) * HL)
                o0 = ps_big.tile([B0, HL], f32, tag="psq")
                o1 = ps_big.tile([B1, HL], f32, tag="psq")
                for j in range(2):
                    js = slice(h * HL + j * Q, h * HL + (j + 1) * Q)
                    nc.tensor.matmul(o0[:, j * Q:(j + 1) * Q],
                                     wT[:, 0:B0], zsb[:, js],
                                     start=True, stop=True)
                    nc.tensor.matmul(o1[:, j * Q:(j + 1) * Q],
                                     wT[:, B0:BAND], zsb[:, js],
                                     start=True, stop=True)
                nc.scalar.copy(osb0[:, hs], o0[:])
                nc.scalar.copy(osb1[:, hs], o1[:])
            nc.sync.dma_start(out=out_d[0:B0, :], in_=osb0[:])
            nc.sync.dma_start(out=out_d[B0:BAND, :], in_=osb1[:])

    nc.compile()
    return nc


def _get_kernel(iters):
    if iters not in _CACHED:
        _CACHED[iters] = _build_kernel(iters)
    return _CACHED[iters]


def kernel(img, k_subspace, p):
    import ml_dtypes
    bf16 = ml_dtypes.bfloat16
    dtype = np.float32
    img = np.asarray(img, dtype)
    p = dtype(np.asarray(p))
    y_w, s0, e, s = _host_prep(img, p)

    D = _dct_mat(ROW)
    eT = np.ascontiguousarray(e.T)
    wT = np.ascontiguousarray((e * (1.0 / s)[:, None]).T)

    iters = int(globals().get("_ITERS", ITERS))
    nc = _get_kernel(iters)

    def bv(x):
        return np.ascontiguousarray(x).astype(bf16)

    in_maps = []
    for c in range(NCORES):
        cs = slice(c * NLOC, (c + 1) * NLOC)
        in_maps.append({
            "yw0": bv(y_w[0:B0, cs]),
            "yw1": bv(y_w[B0:BAND, cs]),
            "s00": bv(s0[0:B0, cs]),
            "s01": bv(s0[B0:BAND, cs]),
            "e": bv(e),
            "e2": bv(2.0 * e),
            "eT": bv(eT),
            "dct": bv(D),
            "dctT": bv(D.T),
            "dloc": bv(D[:, c * RLOC:(c + 1) * RLOC]),
            "wT": bv(wT),
        })

    from concourse.bass_utils import run_bass_kernel_spmd
    res = run_bass_kernel_spmd(nc, in_maps, list(range(NCORES)),
                               trace=bool(globals().get("_TRACE", False)))
    global _LAST_RESULT
    _LAST_RESULT = res
    y_den = np.concatenate([res.results[c]["out"] for c in range(NCORES)],
                           axis=1)
    return np.ascontiguousarray(y_den.T.reshape(ROW, COL, BAND)).astype(dtype)


# revision 36
# speedup vs baseline: 1.1038x; 1.1038x over previous
"""L1HyMixDe denoiser on 8 Trainium2 NeuronCores.

Pipeline: adaptive median + 191x191 noise-whitening/eigendecomposition on host
(tiny LAPACK ops), then the full 40-iteration ADMM (eigen-projection, per-image
2D-DCT soft-threshold denoise, soft-threshold state updates) and the final
reconstruction run on-device, sharded over the spatial axis (2048 pixels/core).
Per iteration the K=10 eigen-images are re-assembled with an AllGather.
Matmul operands and ADMM state are bf16 (PSUM accumulation fp32).
"""
import numpy as np

ROW, COL, BAND = 128, 128, 191
K = 10
MAX_WIN = 7
ITERS = 40
NCORES = 8
NLOC = (ROW * COL) // NCORES          # 2048 pixels per core
RLOC = ROW // NCORES                  # 16 image rows per core
KB = K * COL                          # 1280: batched DCT width
B0, B1 = 128, BAND - 128              # band chunks: 128 + 63
N = ROW * COL
TAU = float(np.sqrt(2.0 * np.log(float(N))))

_CACHED = {}


# ----------------------------------------------------------------- host side

def _dct_mat(n, dtype=np.float32):
    j = np.arange(n)
    k = np.arange(n)[:, None]
    C = np.cos(np.pi * (2 * j[None, :] + 1) * k / (2 * n))
    C *= np.sqrt(2.0 / n)
    C[0] *= np.sqrt(0.5)
    return np.ascontiguousarray(C.astype(dtype))


def _adaptive_median(img):
    """Adaptive median (windows 3,5,7): rad-1 everywhere via np.partition,
    rad-2/3 only at pixels where rad-1 is invalid (ties, mostly edge pad)."""
    H, W, B = img.shape
    r = MAX_WIN // 2
    xp = np.pad(img, ((r, r), (r, r), (0, 0)), mode="edge")
    offs = [(dy - r, dx - r) for dy in range(MAX_WIN) for dx in range(MAX_WIN)]

    def stack(rad, mask=None):
        sel = [i for i, (dy, dx) in enumerate(offs)
               if max(abs(dy), abs(dx)) <= rad]
        views = []
        for i in sel:
            dy, dx = offs[i]
            v = xp[r + dy:r + dy + H, r + dx:r + dx + W]
            views.append(v[mask] if mask is not None else v)
        return np.stack(views, axis=0)

    st = stack(1)
    m = st.shape[0]
    part = np.partition(st, [0, m // 2, m - 1], axis=0)
    zmin, zmed, zmax = part[0], part[m // 2], part[m - 1]
    valid = (zmin < zmed) & (zmed < zmax)
    out = np.where(valid, np.where((zmin < img) & (img < zmax), img, zmed), img)
    done = valid.copy()
    zmed_last = zmed

    bad = ~done
    if bad.any():
        for rad in (2, 3):
            stb = stack(rad, mask=bad)
            m = stb.shape[0]
            part = np.partition(stb, [0, m // 2, m - 1], axis=0)
            zminb, zmedb, zmaxb = part[0], part[m // 2], part[m - 1]
            validb = (zminb < zmedb) & (zmedb < zmaxb)
            imgb = img[bad]
            stageb = np.where((zminb < imgb) & (imgb < zmaxb), imgb, zmedb)
            ob = out[bad]
            out[bad] = np.where(validb & ~done[bad], stageb, ob)
            zl = zmed_last[bad]
            zl[:] = zmedb
            zmed_last[bad] = zl
            done[bad] = done[bad] | validb
            bad = ~done
            if not bad.any():
                break
    return np.where(done, out, zmed_last)


def _host_prep(img, p):
    dtype = np.float32
    img = np.asarray(img, dtype)
    y_og = img.reshape(N, BAND).T
    img_median = _adaptive_median(img)
    img_ro = np.where(np.abs(img - img_median) > p, img_median, img)
    y_ro = img_ro.reshape(N, BAND).T

    eps = dtype(1e-6)
    RR = (y_ro @ y_ro.T).astype(dtype)
    RRi = np.linalg.inv(RR + eps * np.eye(BAND, dtype=dtype)).astype(dtype)
    di = np.diag(RRi)
    M_ = RRi @ RR @ RRi                       # rw_diag without a second data pass
    rw_diag = (np.diag(M_) / (di * di) / N).astype(dtype)

    s = (1.0 / np.sqrt(rw_diag)).astype(dtype)
    y_w = (y_og * s[:, None]).astype(dtype)
    C = (s[:, None] * RR * s[None, :] / N).astype(dtype)
    _, evecs = np.linalg.eigh(C)
    e = np.ascontiguousarray(evecs[:, ::-1][:, :K]).astype(dtype)

    v0 = img_median.reshape(N, BAND).T.astype(dtype)
    s0 = (y_w - v0).astype(dtype)             # s_0 = y - v0 + d0, d0 = 0
    return y_w, s0, e, s


# --------------------------------------------------------------- device side

def _build_kernel(iters):
    import concourse.bass as bass
    import concourse.mybir as mybir
    import concourse.tile as tile
    from concourse import bacc

    f32 = mybir.dt.float32
    bf = mybir.dt.bfloat16
    nc = bacc.Bacc("TRN2", target_bir_lowering=False, debug=False,
                   num_devices=NCORES)

    yw0_d = nc.declare_dram_parameter("yw0", [B0, NLOC], bf, isOutput=False)
    yw1_d = nc.declare_dram_parameter("yw1", [B1, NLOC], bf, isOutput=False)
    s00_d = nc.declare_dram_parameter("s00", [B0, NLOC], bf, isOutput=False)
    s01_d = nc.declare_dram_parameter("s01", [B1, NLOC], bf, isOutput=False)
    e_d = nc.declare_dram_parameter("e", [BAND, K], bf, isOutput=False)
    e2_d = nc.declare_dram_parameter("e2", [BAND, K], bf, isOutput=False)
    eT_d = nc.declare_dram_parameter("eT", [K, BAND], bf, isOutput=False)
    dct_d = nc.declare_dram_parameter("dct", [ROW, ROW], bf, isOutput=False)
    dctT_d = nc.declare_dram_parameter("dctT", [ROW, ROW], bf, isOutput=False)
    dloc_d = nc.declare_dram_parameter("dloc", [ROW, RLOC], bf, isOutput=False)
    wT_d = nc.declare_dram_parameter("wT", [K, BAND], bf, isOutput=False)
    out_d = nc.declare_dram_parameter("out", [BAND, NLOC], f32, isOutput=True)

    NQ = 4              # quarters of NLOC
    Q = NLOC // NQ      # 512

    with tile.TileContext(nc) as tc:
        with (
            tc.tile_pool(name="state", bufs=1) as state,
            tc.tile_pool(name="consts", bufs=1) as consts,
            tc.tile_pool(name="work", bufs=2) as work,
            tc.tile_pool(name="tmp", bufs=3) as tmp,
            tc.tile_pool(name="ps_big", bufs=2, space="PSUM") as ps_big,
            tc.tile_pool(name="ps_dct", bufs=2, space="PSUM") as ps_dct,
            tc.tile_pool(name="dram", bufs=2, space="DRAM") as dram,
        ):
            # ---- persistent state & constants in SBUF (bf16)
            yw0 = state.tile([B0, NLOC], bf, tag="yw0")
            yw1 = state.tile([B1, NLOC], bf, tag="yw1")
            st0 = state.tile([B0, NLOC], bf, tag="st0")
            st1 = state.tile([B1, NLOC], bf, tag="st1")
            dd0 = state.tile([B0, NLOC], bf, tag="dd0")
            dd1 = state.tile([B1, NLOC], bf, tag="dd1")
            t1_0 = state.tile([B0, NLOC], bf, tag="t1_0")
            t1_1 = state.tile([B1, NLOC], bf, tag="t1_1")
            zsb = state.tile([K, NLOC], bf, tag="zsb")
            ybig_a = state.tile([ROW, KB // 2], bf, tag="ybig_a")
            ybig_b = state.tile([ROW, KB // 2], bf, tag="ybig_b")
            eysb = state.tile([K, NLOC], bf, tag="eysb")
            ez0s = state.tile([B0, NLOC], bf, tag="ez0s")
            ez1s = state.tile([B1, NLOC], bf, tag="ez1s")

            e0 = consts.tile([B0, K], bf, tag="e0")
            e1 = consts.tile([B1, K], bf, tag="e1")
            e20 = consts.tile([B0, K], bf, tag="e20")
            e21 = consts.tile([B1, K], bf, tag="e21")
            eT = consts.tile([K, BAND], bf, tag="eT")
            dsb = consts.tile([ROW, ROW], bf, tag="dsb")
            dTsb = consts.tile([ROW, ROW], bf, tag="dTsb")
            dloc = consts.tile([ROW, RLOC], bf, tag="dloc")
            wT = consts.tile([K, BAND], bf, tag="wT")



            nc.sync.dma_start(out=yw0[:], in_=yw0_d[:])
            nc.sync.dma_start(out=yw1[:], in_=yw1_d[:])
            nc.sync.dma_start(out=st0[:], in_=s00_d[:])
            nc.sync.dma_start(out=st1[:], in_=s01_d[:])
            nc.sync.dma_start(out=e0[:], in_=e_d[0:B0, :])
            nc.sync.dma_start(out=e1[:], in_=e_d[B0:BAND, :])
            nc.sync.dma_start(out=e20[:], in_=e2_d[0:B0, :])
            nc.sync.dma_start(out=e21[:], in_=e2_d[B0:BAND, :])
            nc.sync.dma_start(out=eT[:], in_=eT_d[:])
            nc.sync.dma_start(out=dsb[:], in_=dct_d[:])
            nc.sync.dma_start(out=dTsb[:], in_=dctT_d[:])
            nc.sync.dma_start(out=dloc[:], in_=dloc_d[:])
            nc.sync.dma_start(out=wT[:], in_=wT_d[:])
            nc.vector.memset(dd0[:], 0.0)
            nc.vector.memset(dd1[:], 0.0)

            HL = NLOC // 2      # 1024: half width (bf16 moving max)
            dmae = [nc.sync, nc.gpsimd, nc.sync, nc.scalar]

            for t in range(iters):
                # ---- eigen_y = e.T @ s = e.T @ t1 + (2e).T @ d  (K x NLOC)
                ag_in = dram.tile([K, NLOC], bf, tag="ag_in")
                for h in range(2):
                    hs = slice(h * HL, (h + 1) * HL)
                    ey_ps = ps_big.tile([K, HL], f32, tag="psq")
                    for j in range(2):
                        js = slice(h * HL + j * Q, h * HL + (j + 1) * Q)
                        ps = ey_ps[:, j * Q:(j + 1) * Q]
                        if t == 0:
                            nc.tensor.matmul(ps, e0[:], st0[:, js],
                                             start=True, stop=False)
                            nc.tensor.matmul(ps, e1[:], st1[:, js],
                                             start=False, stop=True)
                        else:
                            nc.tensor.matmul(ps, e20[:], dd0[:, js],
                                             start=True, stop=False)
                            nc.tensor.matmul(ps, e21[:], dd1[:, js],
                                             start=False, stop=False)
                            nc.tensor.matmul(ps, e0[:], t1_0[:, js],
                                             start=False, stop=False)
                            nc.tensor.matmul(ps, e1[:], t1_1[:, js],
                                             start=False, stop=True)
                    if h == 0:
                        nc.scalar.copy(eysb[:, hs], ey_ps[:])
                    else:
                        nc.vector.tensor_copy(eysb[:, hs], ey_ps[:])
                    dmae[h].dma_start(out=ag_in[:, hs], in_=eysb[:, hs])

                ag_out = dram.tile([NCORES, K, NLOC], bf, tag="ag_out")
                nc.gpsimd.collective_compute(
                    "AllGather",
                    mybir.AluOpType.bypass,
                    replica_groups=[list(range(NCORES))],
                    ins=[ag_in.opt()],
                    outs=[ag_out.opt()],
                )
                # gather into (row, k*COL + col) layout, group 0 first so its
                # DCT can begin while group 1's DMAs drain
                for g, ybig in ((0, ybig_a), (1, ybig_b)):
                    for rk in range(NCORES):
                        src = ag_out[rk, g * (K // 2):(g + 1) * (K // 2), :]                             .rearrange("k (r w) -> r k w", r=RLOC)
                        dmae[rk % 4].dma_start(
                            out=ybig[rk * RLOC:(rk + 1) * RLOC, :]
                            .rearrange("r (k w) -> r k w", k=K // 2),
                            in_=src,
                        )


                # ---- DCT denoise: two k-groups of 5, pipelined across
                # engines (PE / DVE / ACT stages of group 0 overlap group 1)
                KB2 = KB // 2          # 640
                KH = K // 2            # 5
                for g in range(2):
                    ybig = ybig_a if g == 0 else ybig_b
                    qt_g = ps_dct.tile([ROW, KB2], f32, tag="dctps")
                    for k in range(KH):
                        nc.tensor.matmul(
                            qt_g[:, k * COL:(k + 1) * COL],
                            ybig[:, k * COL:(k + 1) * COL], dTsb[:],
                            start=True, stop=True)
                    qt_sb = work.tile([ROW, KB2], bf, tag="qt_sb")
                    nc.vector.tensor_copy(qt_sb[:], qt_g[:])

                    ct_g = ps_dct.tile([ROW, KB2], f32, tag="dctps")
                    nc.tensor.matmul(ct_g[:, 0:512], dTsb[:], qt_sb[:, 0:512],
                                     start=True, stop=True)
                    nc.tensor.matmul(ct_g[:, 512:KB2], dTsb[:],
                                     qt_sb[:, 512:KB2],
                                     start=True, stop=True)
                    ct_sb = tmp.tile([ROW, KB2], bf, tag="ct_sb")
                    nc.scalar.copy(ct_sb[:], ct_g[:])
                    cl_sb = tmp.tile([ROW, KB2], bf, tag="cl_sb")
                    nc.vector.tensor_scalar_max(cl_sb[:], ct_sb[:], -TAU)
                    cl2_sb = tmp.tile([ROW, KB2], bf, tag="cl2_sb")
                    nc.vector.tensor_scalar_min(cl2_sb[:], cl_sb[:], TAU)
                    cpt_sb = work.tile([ROW, KB2], bf, tag="cpt_sb")
                    nc.vector.tensor_sub(cpt_sb[:], ct_sb[:], cl2_sb[:])

                    n1_g = ps_dct.tile([ROW, KB2], f32, tag="dctps")
                    for k in range(KH):
                        nc.tensor.matmul(
                            n1_g[:, k * COL:(k + 1) * COL],
                            cpt_sb[:, k * COL:(k + 1) * COL], dsb[:],
                            start=True, stop=True)
                    n1_sb = work.tile([ROW, KB2], bf, tag="n1_sb")
                    nc.vector.tensor_copy(n1_sb[:], n1_g[:])

                    zl_g = ps_dct.tile([RLOC, KB2], f32, tag="dctps")
                    nc.tensor.matmul(zl_g[:, 0:512], dloc[:], n1_sb[:, 0:512],
                                     start=True, stop=True)
                    nc.tensor.matmul(zl_g[:, 512:KB2], dloc[:],
                                     n1_sb[:, 512:KB2],
                                     start=True, stop=True)
                    zs_sb = tmp.tile([RLOC, KB2], bf, tag="zs_sb")
                    for k in range(KH):
                        ks = slice(k * COL, (k + 1) * COL)
                        kg = g * KH + k
                        if k % 2 == 0:
                            nc.scalar.copy(zs_sb[:, ks], zl_g[:, ks])
                        else:
                            nc.vector.tensor_copy(zs_sb[:, ks], zl_g[:, ks])
                        dmae[kg % 4].dma_start(
                            out=zsb[kg:kg + 1, :]
                            .rearrange("a (r w) -> a r w", r=RLOC),
                            in_=zs_sb[:, ks],
                        )


                if t == iters - 1:
                    break

                # ---- EZ = e @ z ; state updates pipelined in halves
                # t1 = EZ - d; a = y - t1; d' = clip(a)
                for h in range(2):
                    hs = slice(h * HL, (h + 1) * HL)
                    ez0 = ps_big.tile([B0, HL], f32, tag="psq")
                    ez1 = ps_big.tile([B1, HL], f32, tag="psq")
                    for j in range(2):
                        js = slice(h * HL + j * Q, h * HL + (j + 1) * Q)
                        nc.tensor.matmul(ez0[:, j * Q:(j + 1) * Q],
                                         eT[:, 0:B0], zsb[:, js],
                                         start=True, stop=True)
                        nc.tensor.matmul(ez1[:, j * Q:(j + 1) * Q],
                                         eT[:, B0:BAND], zsb[:, js],
                                         start=True, stop=True)
                    nc.scalar.copy(ez0s[:, hs], ez0[:])
                    nc.scalar.copy(ez1s[:, hs], ez1[:])
                    for (ezs, ddt, yyt, t1t) in (
                        (ez0s, dd0, yw0, t1_0),
                        (ez1s, dd1, yw1, t1_1),
                    ):
                        P = ddt.shape[0]
                        a_ = tmp.tile([P, HL], bf, tag="a_")
                        b_ = tmp.tile([P, HL], bf, tag="b_")
                        nc.vector.tensor_sub(t1t[:, hs], ezs[:, hs], ddt[:, hs])
                        nc.vector.tensor_sub(a_[:], yyt[:, hs], t1t[:, hs])
                        nc.vector.tensor_scalar_max(b_[:], a_[:], -1.0)
                        nc.vector.tensor_scalar_min(ddt[:, hs], b_[:], 1.0)


            # ---- reconstruction: out = W @ z
            osb0 = state.tile([B0, NLOC], f32, tag="osb0")
            osb1 = state.tile([B1, NLOC], f32, tag="osb1")
            for h in range(2):
                hs = slice(h * HL, (h + 1) * HL)
                o0 = ps_big.tile([B0, HL], f32, tag="psq")
                o1 = ps_big.tile([B1, HL], f32, tag="psq")
                for j in range(2):
                    js = slice(h * HL + j * Q, h * HL + (j + 1) * Q)
                    nc.tensor.matmul(o0[:, j * Q:(j + 1) * Q],
                                     wT[:, 0:B0], zsb[:, js],
                                     start=True, stop=True)
                    nc.tensor.matmul(o1[:, j * Q:(j + 1) * Q],
                                     wT[:, B0:BAND], zsb[:, js],
                                     start=True, stop=True)
                nc.scalar.copy(osb0[:, hs], o0[:])
                nc.scalar.copy(osb1[:, hs], o1[:])
            nc.sync.dma_start(out=out_d[0:B0, :], in_=osb0[:])
            nc.sync.dma_start(out=out_d[B0:BAND, :], in_=osb1[:])

    nc.compile()
    return nc


def _get_kernel(iters):
    if iters not in _CACHED:
        _CACHED[iters] = _build_kernel(iters)
    return _CACHED[iters]


def kernel(img, k_subspace, p):
    import ml_dtypes
    bf16 = ml_dtypes.bfloat16
    dtype = np.float32
    img = np.asarray(img, dtype)
    p = dtype(np.asarray(p))
    y_w, s0, e, s = _host_prep(img, p)

    D = _dct_mat(ROW)
    eT = np.ascontiguousarray(e.T)
    wT = np.ascontiguousarray((e * (1.0 / s)[:, None]).T)

    iters = int(globals().get("_ITERS", ITERS))
    nc = _get_kernel(iters)

    def bv(x):
        return np.ascontiguousarray(x).astype(bf16)

    in_maps = []
    for c in range(NCORES):
        cs = slice(c * NLOC, (c + 1) * NLOC)
        in_maps.append({
            "yw0": bv(y_w[0:B0, cs]),
            "yw1": bv(y_w[B0:BAND, cs]),
            "s00": bv(s0[0:B0, cs]),
            "s01": bv(s0[B0:BAND, cs]),
            "e": bv(e),
            "e2": bv(2.0 * e),
            "eT": bv(eT),
            "dct": bv(D),
            "dctT": bv(D.T),
            "dloc": bv(D[:, c * RLOC:(c + 1) * RLOC]),
            "wT": bv(wT),
        })

    from concourse.bass_utils import run_bass_kernel_spmd
    res = run_bass_kernel_spmd(nc, in_maps, list(range(NCORES)),
                               trace=bool(globals().get("_TRACE", False)))
    global _LAST_RESULT
    _LAST_RESULT = res
    y_den = np.concatenate([res.results[c]["out"] for c in range(NCORES)],
                           axis=1)
    return np.ascontiguousarray(y_den.T.reshape(ROW, COL, BAND)).astype(dtype)


# revision 37
# speedup vs baseline: 1.1295x; 1.0233x over previous
"""L1HyMixDe denoiser on 8 Trainium2 NeuronCores.

Pipeline: adaptive median + 191x191 noise-whitening/eigendecomposition on host
(tiny LAPACK ops), then the full 40-iteration ADMM (eigen-projection, per-image
2D-DCT soft-threshold denoise, soft-threshold state updates) and the final
reconstruction run on-device, sharded over the spatial axis (2048 pixels/core).
Per iteration the K=10 eigen-images are re-assembled with an AllGather.
Matmul operands and ADMM state are bf16 (PSUM accumulation fp32).
"""
import numpy as np

ROW, COL, BAND = 128, 128, 191
K = 10
MAX_WIN = 7
ITERS = 40
NCORES = 8
NLOC = (ROW * COL) // NCORES          # 2048 pixels per core
RLOC = ROW // NCORES                  # 16 image rows per core
KB = K * COL                          # 1280: batched DCT width
B0, B1 = 128, BAND - 128              # band chunks: 128 + 63
N = ROW * COL
TAU = float(np.sqrt(2.0 * np.log(float(N))))

_CACHED = {}


# ----------------------------------------------------------------- host side

def _dct_mat(n, dtype=np.float32):
    j = np.arange(n)
    k = np.arange(n)[:, None]
    C = np.cos(np.pi * (2 * j[None, :] + 1) * k / (2 * n))
    C *= np.sqrt(2.0 / n)
    C[0] *= np.sqrt(0.5)
    return np.ascontiguousarray(C.astype(dtype))


def _adaptive_median(img):
    """Adaptive median (windows 3,5,7): rad-1 everywhere via np.partition,
    rad-2/3 only at pixels where rad-1 is invalid (ties, mostly edge pad)."""
    H, W, B = img.shape
    r = MAX_WIN // 2
    xp = np.pad(img, ((r, r), (r, r), (0, 0)), mode="edge")
    offs = [(dy - r, dx - r) for dy in range(MAX_WIN) for dx in range(MAX_WIN)]

    def stack(rad, mask=None):
        sel = [i for i, (dy, dx) in enumerate(offs)
               if max(abs(dy), abs(dx)) <= rad]
        views = []
        for i in sel:
            dy, dx = offs[i]
            v = xp[r + dy:r + dy + H, r + dx:r + dx + W]
            views.append(v[mask] if mask is not None else v)
        return np.stack(views, axis=0)

    st = stack(1)
    m = st.shape[0]
    part = np.partition(st, [0, m // 2, m - 1], axis=0)
    zmin, zmed, zmax = part[0], part[m // 2], part[m - 1]
    valid = (zmin < zmed) & (zmed < zmax)
    out = np.where(valid, np.where((zmin < img) & (img < zmax), img, zmed), img)
    done = valid.copy()
    zmed_last = zmed

    bad = ~done
    if bad.any():
        for rad in (2, 3):
            stb = stack(rad, mask=bad)
            m = stb.shape[0]
            part = np.partition(stb, [0, m // 2, m - 1], axis=0)
            zminb, zmedb, zmaxb = part[0], part[m // 2], part[m - 1]
            validb = (zminb < zmedb) & (zmedb < zmaxb)
            imgb = img[bad]
            stageb = np.where((zminb < imgb) & (imgb < zmaxb), imgb, zmedb)
            ob = out[bad]
            out[bad] = np.where(validb & ~done[bad], stageb, ob)
            zl = zmed_last[bad]
            zl[:] = zmedb
            zmed_last[bad] = zl
            done[bad] = done[bad] | validb
            bad = ~done
            if not bad.any():
                break
    return np.where(done, out, zmed_last)


def _host_prep(img, p):
    dtype = np.float32
    img = np.asarray(img, dtype)
    y_og = img.reshape(N, BAND).T
    img_median = _adaptive_median(img)
    img_ro = np.where(np.abs(img - img_median) > p, img_median, img)
    y_ro = img_ro.reshape(N, BAND).T

    eps = dtype(1e-6)
    RR = (y_ro @ y_ro.T).astype(dtype)
    RRi = np.linalg.inv(RR + eps * np.eye(BAND, dtype=dtype)).astype(dtype)
    di = np.diag(RRi)
    M_ = RRi @ RR @ RRi                       # rw_diag without a second data pass
    rw_diag = (np.diag(M_) / (di * di) / N).astype(dtype)

    s = (1.0 / np.sqrt(rw_diag)).astype(dtype)
    y_w = (y_og * s[:, None]).astype(dtype)
    C = (s[:, None] * RR * s[None, :] / N).astype(dtype)
    _, evecs = np.linalg.eigh(C)
    e = np.ascontiguousarray(evecs[:, ::-1][:, :K]).astype(dtype)

    v0 = img_median.reshape(N, BAND).T.astype(dtype)
    s0 = (y_w - v0).astype(dtype)             # s_0 = y - v0 + d0, d0 = 0
    return y_w, s0, e, s


# --------------------------------------------------------------- device side

def _build_kernel(iters):
    import concourse.bass as bass
    import concourse.mybir as mybir
    import concourse.tile as tile
    from concourse import bacc

    f32 = mybir.dt.float32
    bf = mybir.dt.bfloat16
    nc = bacc.Bacc("TRN2", target_bir_lowering=False, debug=False,
                   num_devices=NCORES)

    yw0_d = nc.declare_dram_parameter("yw0", [B0, NLOC], bf, isOutput=False)
    yw1_d = nc.declare_dram_parameter("yw1", [B1, NLOC], bf, isOutput=False)
    s00_d = nc.declare_dram_parameter("s00", [B0, NLOC], bf, isOutput=False)
    s01_d = nc.declare_dram_parameter("s01", [B1, NLOC], bf, isOutput=False)
    e_d = nc.declare_dram_parameter("e", [BAND, K], bf, isOutput=False)
    e2_d = nc.declare_dram_parameter("e2", [BAND, K], bf, isOutput=False)
    eT_d = nc.declare_dram_parameter("eT", [K, BAND], bf, isOutput=False)
    dct_d = nc.declare_dram_parameter("dct", [ROW, ROW], bf, isOutput=False)
    dctT_d = nc.declare_dram_parameter("dctT", [ROW, ROW], bf, isOutput=False)
    dloc_d = nc.declare_dram_parameter("dloc", [ROW, RLOC], bf, isOutput=False)
    wT_d = nc.declare_dram_parameter("wT", [K, BAND], bf, isOutput=False)
    out_d = nc.declare_dram_parameter("out", [BAND, NLOC], f32, isOutput=True)

    NQ = 4              # quarters of NLOC
    Q = NLOC // NQ      # 512

    with tile.TileContext(nc) as tc:
        with (
            tc.tile_pool(name="state", bufs=1) as state,
            tc.tile_pool(name="consts", bufs=1) as consts,
            tc.tile_pool(name="work", bufs=2) as work,
            tc.tile_pool(name="tmp", bufs=3) as tmp,
            tc.tile_pool(name="ps_big", bufs=2, space="PSUM") as ps_big,
            tc.tile_pool(name="ps_dct", bufs=2, space="PSUM") as ps_dct,
            tc.tile_pool(name="dram", bufs=2, space="DRAM") as dram,
        ):
            # ---- persistent state & constants in SBUF (bf16)
            yw0 = state.tile([B0, NLOC], bf, tag="yw0")
            yw1 = state.tile([B1, NLOC], bf, tag="yw1")
            st0 = state.tile([B0, NLOC], bf, tag="st0")
            st1 = state.tile([B1, NLOC], bf, tag="st1")
            dd0 = state.tile([B0, NLOC], bf, tag="dd0")
            dd1 = state.tile([B1, NLOC], bf, tag="dd1")
            t1_0 = state.tile([B0, NLOC], bf, tag="t1_0")
            t1_1 = state.tile([B1, NLOC], bf, tag="t1_1")
            zsb = state.tile([K, NLOC], bf, tag="zsb")
            ybig_a = state.tile([ROW, KB // 2], bf, tag="ybig_a")
            ybig_b = state.tile([ROW, KB // 2], bf, tag="ybig_b")
            eysb = state.tile([K, NLOC], bf, tag="eysb")
            ez0s = state.tile([B0, NLOC], bf, tag="ez0s")
            ez1s = state.tile([B1, NLOC], bf, tag="ez1s")

            e0 = consts.tile([B0, K], bf, tag="e0")
            e1 = consts.tile([B1, K], bf, tag="e1")
            e20 = consts.tile([B0, K], bf, tag="e20")
            e21 = consts.tile([B1, K], bf, tag="e21")
            eT = consts.tile([K, BAND], bf, tag="eT")
            dsb = consts.tile([ROW, ROW], bf, tag="dsb")
            dTsb = consts.tile([ROW, ROW], bf, tag="dTsb")
            dloc = consts.tile([ROW, RLOC], bf, tag="dloc")
            wT = consts.tile([K, BAND], bf, tag="wT")



            nc.sync.dma_start(out=yw0[:], in_=yw0_d[:])
            nc.sync.dma_start(out=yw1[:], in_=yw1_d[:])
            nc.sync.dma_start(out=st0[:], in_=s00_d[:])
            nc.sync.dma_start(out=st1[:], in_=s01_d[:])
            nc.sync.dma_start(out=e0[:], in_=e_d[0:B0, :])
            nc.sync.dma_start(out=e1[:], in_=e_d[B0:BAND, :])
            nc.sync.dma_start(out=e20[:], in_=e2_d[0:B0, :])
            nc.sync.dma_start(out=e21[:], in_=e2_d[B0:BAND, :])
            nc.sync.dma_start(out=eT[:], in_=eT_d[:])
            nc.sync.dma_start(out=dsb[:], in_=dct_d[:])
            nc.sync.dma_start(out=dTsb[:], in_=dctT_d[:])
            nc.sync.dma_start(out=dloc[:], in_=dloc_d[:])
            nc.sync.dma_start(out=wT[:], in_=wT_d[:])
            nc.vector.memset(dd0[:], 0.0)
            nc.vector.memset(dd1[:], 0.0)

            HL = NLOC // 2      # 1024: half width (bf16 moving max)
            dmae = [nc.sync, nc.gpsimd, nc.sync, nc.scalar]

            for t in range(iters):
                # ---- eigen_y = e.T @ s = e.T @ t1 + (2e).T @ d  (K x NLOC)
                ag_in = dram.tile([K, NLOC], bf, tag="ag_in")
                for h in range(2):
                    hs = slice(h * HL, (h + 1) * HL)
                    ey_ps = ps_big.tile([K, HL], f32, tag="psq")
                    for j in range(2):
                        js = slice(h * HL + j * Q, h * HL + (j + 1) * Q)
                        ps = ey_ps[:, j * Q:(j + 1) * Q]
                        if t == 0:
                            nc.tensor.matmul(ps, e0[:], st0[:, js],
                                             start=True, stop=False)
                            nc.tensor.matmul(ps, e1[:], st1[:, js],
                                             start=False, stop=True)
                        else:
                            nc.tensor.matmul(ps, e20[:], dd0[:, js],
                                             start=True, stop=False)
                            nc.tensor.matmul(ps, e21[:], dd1[:, js],
                                             start=False, stop=False)
                            nc.tensor.matmul(ps, e0[:], t1_0[:, js],
                                             start=False, stop=False)
                            nc.tensor.matmul(ps, e1[:], t1_1[:, js],
                                             start=False, stop=True)
                    if h == 0:
                        nc.scalar.copy(eysb[:, hs], ey_ps[:])
                    else:
                        nc.vector.tensor_copy(eysb[:, hs], ey_ps[:])
                    dmae[h].dma_start(out=ag_in[:, hs], in_=eysb[:, hs])

                ag_out = dram.tile([NCORES, K, NLOC], bf, tag="ag_out")
                nc.gpsimd.collective_compute(
                    "AllGather",
                    mybir.AluOpType.bypass,
                    replica_groups=[list(range(NCORES))],
                    ins=[ag_in.opt()],
                    outs=[ag_out.opt()],
                )
                # gather into (row, k*COL + col) layout, group 0 first so its
                # DCT can begin while group 1's DMAs drain
                for g, ybig in ((0, ybig_a), (1, ybig_b)):
                    for rk in range(NCORES):
                        src = ag_out[rk, g * (K // 2):(g + 1) * (K // 2), :]                             .rearrange("k (r w) -> r k w", r=RLOC)
                        dmae[rk % 4].dma_start(
                            out=ybig[rk * RLOC:(rk + 1) * RLOC, :]
                            .rearrange("r (k w) -> r k w", k=K // 2),
                            in_=src,
                        )


                # ---- DCT denoise: two k-groups of 5, pipelined across
                # engines (PE / DVE / ACT stages of group 0 overlap group 1)
                KB2 = KB // 2          # 640
                KH = K // 2            # 5
                for g in range(2):
                    ybig = ybig_a if g == 0 else ybig_b
                    qt_g = ps_dct.tile([ROW, KB2], f32, tag="dctps")
                    for k in range(KH):
                        nc.tensor.matmul(
                            qt_g[:, k * COL:(k + 1) * COL],
                            ybig[:, k * COL:(k + 1) * COL], dTsb[:],
                            start=True, stop=True)
                    qt_sb = work.tile([ROW, KB2], bf, tag="qt_sb")
                    nc.vector.tensor_copy(qt_sb[:], qt_g[:])

                    ct_g = ps_dct.tile([ROW, KB2], f32, tag="dctps")
                    nc.tensor.matmul(ct_g[:, 0:512], dTsb[:], qt_sb[:, 0:512],
                                     start=True, stop=True)
                    nc.tensor.matmul(ct_g[:, 512:KB2], dTsb[:],
                                     qt_sb[:, 512:KB2],
                                     start=True, stop=True)
                    ct_sb = tmp.tile([ROW, KB2], bf, tag="ct_sb")
                    nc.scalar.copy(ct_sb[:], ct_g[:])
                    cl_sb = tmp.tile([ROW, KB2], bf, tag="cl_sb")
                    nc.vector.tensor_scalar_max(cl_sb[:], ct_sb[:], -TAU)
                    cl2_sb = tmp.tile([ROW, KB2], bf, tag="cl2_sb")
                    nc.vector.tensor_scalar_min(cl2_sb[:], cl_sb[:], TAU)
                    cpt_sb = work.tile([ROW, KB2], bf, tag="cpt_sb")
                    nc.vector.tensor_sub(cpt_sb[:], ct_sb[:], cl2_sb[:])

                    n1_g = ps_dct.tile([ROW, KB2], f32, tag="dctps")
                    for k in range(KH):
                        nc.tensor.matmul(
                            n1_g[:, k * COL:(k + 1) * COL],
                            cpt_sb[:, k * COL:(k + 1) * COL], dsb[:],
                            start=True, stop=True)
                    n1_sb = work.tile([ROW, KB2], bf, tag="n1_sb")
                    nc.vector.tensor_copy(n1_sb[:], n1_g[:])

                    zl_g = ps_big.tile([RLOC, KB2], f32, tag="psq")
                    nc.tensor.matmul(zl_g[:, 0:512], dloc[:], n1_sb[:, 0:512],
                                     start=True, stop=True)
                    nc.tensor.matmul(zl_g[:, 512:KB2], dloc[:],
                                     n1_sb[:, 512:KB2],
                                     start=True, stop=True)
                    zs_sb = tmp.tile([RLOC, KB2], bf, tag="zs_sb")
                    for k in range(KH):
                        ks = slice(k * COL, (k + 1) * COL)
                        kg = g * KH + k
                        if k % 2 == 0:
                            nc.scalar.copy(zs_sb[:, ks], zl_g[:, ks])
                        else:
                            nc.vector.tensor_copy(zs_sb[:, ks], zl_g[:, ks])
                        dmae[kg % 4].dma_start(
                            out=zsb[kg:kg + 1, :]
                            .rearrange("a (r w) -> a r w", r=RLOC),
                            in_=zs_sb[:, ks],
                        )


                if t == iters - 1:
                    break

                # ---- EZ = e @ z ; state updates pipelined in halves
                # t1 = EZ - d; a = y - t1; d' = clip(a)
                for h in range(2):
                    hs = slice(h * HL, (h + 1) * HL)
                    ez0 = ps_big.tile([B0, HL], f32, tag="psq")
                    ez1 = ps_big.tile([B1, HL], f32, tag="psq")
                    for j in range(2):
                        js = slice(h * HL + j * Q, h * HL + (j + 1) * Q)
                        nc.tensor.matmul(ez0[:, j * Q:(j + 1) * Q],
                                         eT[:, 0:B0], zsb[:, js],
                                         start=True, stop=True)
                        nc.tensor.matmul(ez1[:, j * Q:(j + 1) * Q],
                                         eT[:, B0:BAND], zsb[:, js],
                                         start=True, stop=True)
                    nc.scalar.copy(ez0s[:, hs], ez0[:])
                    nc.scalar.copy(ez1s[:, hs], ez1[:])
                    for (ezs, ddt, yyt, t1t) in (
                        (ez0s, dd0, yw0, t1_0),
                        (ez1s, dd1, yw1, t1_1),
                    ):
                        P = ddt.shape[0]
                        a_ = tmp.tile([P, HL], bf, tag="a_")
                        b_ = tmp.tile([P, HL], bf, tag="b_")
                        nc.vector.tensor_sub(t1t[:, hs], ezs[:, hs], ddt[:, hs])
                        nc.vector.tensor_sub(a_[:], yyt[:, hs], t1t[:, hs])
                        nc.vector.tensor_scalar_max(b_[:], a_[:], -1.0)
                        nc.vector.tensor_scalar_min(ddt[:, hs], b_[:], 1.0)


            # ---- reconstruction: out = W @ z
            osb0 = state.tile([B0, NLOC], f32, tag="osb0")
            osb1 = state.tile([B1, NLOC], f32, tag="osb1")
            for h in range(2):
                hs = slice(h * HL, (h + 1) * HL)
                o0 = ps_big.tile([B0, HL], f32, tag="psq")
                o1 = ps_big.tile([B1, HL], f32, tag="psq")
                for j in range(2):
                    js = slice(h * HL + j * Q, h * HL + (j + 1) * Q)
                    nc.tensor.matmul(o0[:, j * Q:(j + 1) * Q],
                                     wT[:, 0:B0], zsb[:, js],
                                     start=True, stop=True)
                    nc.tensor.matmul(o1[:, j * Q:(j + 1) * Q],
                                     wT[:, B0:BAND], zsb[:, js],
                                     start=True, stop=True)
                nc.scalar.copy(osb0[:, hs], o0[:])
                nc.scalar.copy(osb1[:, hs], o1[:])
            nc.sync.dma_start(out=out_d[0:B0, :], in_=osb0[:])
            nc.sync.dma_start(out=out_d[B0:BAND, :], in_=osb1[:])

    nc.compile()
    return nc


def _get_kernel(iters):
    if iters not in _CACHED:
        _CACHED[iters] = _build_kernel(iters)
    return _CACHED[iters]


def kernel(img, k_subspace, p):
    import ml_dtypes
    bf16 = ml_dtypes.bfloat16
    dtype = np.float32
    img = np.asarray(img, dtype)
    p = dtype(np.asarray(p))
    y_w, s0, e, s = _host_prep(img, p)

    D = _dct_mat(ROW)
    eT = np.ascontiguousarray(e.T)
    wT = np.ascontiguousarray((e * (1.0 / s)[:, None]).T)

    iters = int(globals().get("_ITERS", ITERS))
    nc = _get_kernel(iters)

    def bv(x):
        return np.ascontiguousarray(x).astype(bf16)

    in_maps = []
    for c in range(NCORES):
        cs = slice(c * NLOC, (c + 1) * NLOC)
        in_maps.append({
            "yw0": bv(y_w[0:B0, cs]),
            "yw1": bv(y_w[B0:BAND, cs]),
            "s00": bv(s0[0:B0, cs]),
            "s01": bv(s0[B0:BAND, cs]),
            "e": bv(e),
            "e2": bv(2.0 * e),
            "eT": bv(eT),
            "dct": bv(D),
            "dctT": bv(D.T),
            "dloc": bv(D[:, c * RLOC:(c + 1) * RLOC]),
            "wT": bv(wT),
        })

    from concourse.bass_utils import run_bass_kernel_spmd
    res = run_bass_kernel_spmd(nc, in_maps, list(range(NCORES)),
                               trace=bool(globals().get("_TRACE", False)))
    global _LAST_RESULT
    _LAST_RESULT = res
    y_den = np.concatenate([res.results[c]["out"] for c in range(NCORES)],
                           axis=1)
    return np.ascontiguousarray(y_den.T.reshape(ROW, COL, BAND)).astype(dtype)


# revision 38
# speedup vs baseline: 1.1689x; 1.0349x over previous
"""L1HyMixDe denoiser on 8 Trainium2 NeuronCores.

Pipeline: adaptive median + 191x191 noise-whitening/eigendecomposition on host
(tiny LAPACK ops), then the full 40-iteration ADMM (eigen-projection, per-image
2D-DCT soft-threshold denoise, soft-threshold state updates) and the final
reconstruction run on-device, sharded over the spatial axis (2048 pixels/core).
Per iteration the K=10 eigen-images are re-assembled with an AllGather.
Matmul operands and ADMM state are bf16 (PSUM accumulation fp32).
"""
import numpy as np

ROW, COL, BAND = 128, 128, 191
K = 10
MAX_WIN = 7
ITERS = 40
NCORES = 8
NLOC = (ROW * COL) // NCORES          # 2048 pixels per core
RLOC = ROW // NCORES                  # 16 image rows per core
KB = K * COL                          # 1280: batched DCT width
B0, B1 = 128, BAND - 128              # band chunks: 128 + 63
N = ROW * COL
TAU = float(np.sqrt(2.0 * np.log(float(N))))

_CACHED = {}


# ----------------------------------------------------------------- host side

def _dct_mat(n, dtype=np.float32):
    j = np.arange(n)
    k = np.arange(n)[:, None]
    C = np.cos(np.pi * (2 * j[None, :] + 1) * k / (2 * n))
    C *= np.sqrt(2.0 / n)
    C[0] *= np.sqrt(0.5)
    return np.ascontiguousarray(C.astype(dtype))


def _adaptive_median(img):
    """Adaptive median (windows 3,5,7): rad-1 everywhere via np.partition,
    rad-2/3 only at pixels where rad-1 is invalid (ties, mostly edge pad)."""
    H, W, B = img.shape
    r = MAX_WIN // 2
    xp = np.pad(img, ((r, r), (r, r), (0, 0)), mode="edge")
    offs = [(dy - r, dx - r) for dy in range(MAX_WIN) for dx in range(MAX_WIN)]

    def stack(rad, mask=None):
        sel = [i for i, (dy, dx) in enumerate(offs)
               if max(abs(dy), abs(dx)) <= rad]
        views = []
        for i in sel:
            dy, dx = offs[i]
            v = xp[r + dy:r + dy + H, r + dx:r + dx + W]
            views.append(v[mask] if mask is not None else v)
        return np.stack(views, axis=0)

    st = stack(1)
    m = st.shape[0]
    part = np.partition(st, [0, m // 2, m - 1], axis=0)
    zmin, zmed, zmax = part[0], part[m // 2], part[m - 1]
    valid = (zmin < zmed) & (zmed < zmax)
    out = np.where(valid, np.where((zmin < img) & (img < zmax), img, zmed), img)
    done = valid.copy()
    zmed_last = zmed

    bad = ~done
    if bad.any():
        for rad in (2, 3):
            stb = stack(rad, mask=bad)
            m = stb.shape[0]
            part = np.partition(stb, [0, m // 2, m - 1], axis=0)
            zminb, zmedb, zmaxb = part[0], part[m // 2], part[m - 1]
            validb = (zminb < zmedb) & (zmedb < zmaxb)
            imgb = img[bad]
            stageb = np.where((zminb < imgb) & (imgb < zmaxb), imgb, zmedb)
            ob = out[bad]
            out[bad] = np.where(validb & ~done[bad], stageb, ob)
            zl = zmed_last[bad]
            zl[:] = zmedb
            zmed_last[bad] = zl
            done[bad] = done[bad] | validb
            bad = ~done
            if not bad.any():
                break
    return np.where(done, out, zmed_last)


def _host_prep(img, p):
    dtype = np.float32
    img = np.asarray(img, dtype)
    y_og = img.reshape(N, BAND).T
    img_median = _adaptive_median(img)
    img_ro = np.where(np.abs(img - img_median) > p, img_median, img)
    y_ro = img_ro.reshape(N, BAND).T

    eps = dtype(1e-6)
    RR = (y_ro @ y_ro.T).astype(dtype)
    RRi = np.linalg.inv(RR + eps * np.eye(BAND, dtype=dtype)).astype(dtype)
    di = np.diag(RRi)
    M_ = RRi @ RR @ RRi                       # rw_diag without a second data pass
    rw_diag = (np.diag(M_) / (di * di) / N).astype(dtype)

    s = (1.0 / np.sqrt(rw_diag)).astype(dtype)
    y_w = (y_og * s[:, None]).astype(dtype)
    C = (s[:, None] * RR * s[None, :] / N).astype(dtype)
    _, evecs = np.linalg.eigh(C)
    e = np.ascontiguousarray(evecs[:, ::-1][:, :K]).astype(dtype)

    v0 = img_median.reshape(N, BAND).T.astype(dtype)
    s0 = (y_w - v0).astype(dtype)             # s_0 = y - v0 + d0, d0 = 0
    return y_w, s0, e, s


# --------------------------------------------------------------- device side

def _build_kernel(iters):
    import concourse.bass as bass
    import concourse.mybir as mybir
    import concourse.tile as tile
    from concourse import bacc

    f32 = mybir.dt.float32
    bf = mybir.dt.bfloat16
    nc = bacc.Bacc("TRN2", target_bir_lowering=False, debug=False,
                   num_devices=NCORES)

    yw0_d = nc.declare_dram_parameter("yw0", [B0, NLOC], bf, isOutput=False)
    yw1_d = nc.declare_dram_parameter("yw1", [B1, NLOC], bf, isOutput=False)
    s00_d = nc.declare_dram_parameter("s00", [B0, NLOC], bf, isOutput=False)
    s01_d = nc.declare_dram_parameter("s01", [B1, NLOC], bf, isOutput=False)
    e_d = nc.declare_dram_parameter("e", [BAND, K], bf, isOutput=False)
    e2_d = nc.declare_dram_parameter("e2", [BAND, K], bf, isOutput=False)
    eT_d = nc.declare_dram_parameter("eT", [K, BAND], bf, isOutput=False)
    dct_d = nc.declare_dram_parameter("dct", [ROW, ROW], bf, isOutput=False)
    dctT_d = nc.declare_dram_parameter("dctT", [ROW, ROW], bf, isOutput=False)
    dloc_d = nc.declare_dram_parameter("dloc", [ROW, RLOC], bf, isOutput=False)
    wT_d = nc.declare_dram_parameter("wT", [K, BAND], bf, isOutput=False)
    out_d = nc.declare_dram_parameter("out", [BAND, NLOC], f32, isOutput=True)

    NQ = 4              # quarters of NLOC
    Q = NLOC // NQ      # 512

    with tile.TileContext(nc) as tc:
        with (
            tc.tile_pool(name="state", bufs=1) as state,
            tc.tile_pool(name="consts", bufs=1) as consts,
            tc.tile_pool(name="work", bufs=2) as work,
            tc.tile_pool(name="tmp", bufs=3) as tmp,
            tc.tile_pool(name="ps_big", bufs=2, space="PSUM") as ps_big,
            tc.tile_pool(name="ps_dct", bufs=2, space="PSUM") as ps_dct,
            tc.tile_pool(name="dram", bufs=2, space="DRAM") as dram,
        ):
            # ---- persistent state & constants in SBUF (bf16)
            yw0 = state.tile([B0, NLOC], bf, tag="yw0")
            yw1 = state.tile([B1, NLOC], bf, tag="yw1")
            st0 = state.tile([B0, NLOC], bf, tag="st0")
            st1 = state.tile([B1, NLOC], bf, tag="st1")
            dd0 = state.tile([B0, NLOC], bf, tag="dd0")
            dd1 = state.tile([B1, NLOC], bf, tag="dd1")
            t1_0 = state.tile([B0, NLOC], bf, tag="t1_0")
            t1_1 = state.tile([B1, NLOC], bf, tag="t1_1")
            zsb = state.tile([K, NLOC], bf, tag="zsb")
            ybig_a = state.tile([ROW, KB // 2], bf, tag="ybig_a")
            ybig_b = state.tile([ROW, KB // 2], bf, tag="ybig_b")
            eysb = state.tile([K, NLOC], bf, tag="eysb")
            ez0s = state.tile([B0, NLOC], bf, tag="ez0s")
            ez1s = state.tile([B1, NLOC], bf, tag="ez1s")

            e0 = consts.tile([B0, K], bf, tag="e0")
            e1 = consts.tile([B1, K], bf, tag="e1")
            e20 = consts.tile([B0, K], bf, tag="e20")
            e21 = consts.tile([B1, K], bf, tag="e21")
            eT = consts.tile([K, BAND], bf, tag="eT")
            dsb = consts.tile([ROW, ROW], bf, tag="dsb")
            dTsb = consts.tile([ROW, ROW], bf, tag="dTsb")
            dloc = consts.tile([ROW, RLOC], bf, tag="dloc")
            wT = consts.tile([K, BAND], bf, tag="wT")



            nc.sync.dma_start(out=yw0[:], in_=yw0_d[:])
            nc.sync.dma_start(out=yw1[:], in_=yw1_d[:])
            nc.sync.dma_start(out=st0[:], in_=s00_d[:])
            nc.sync.dma_start(out=st1[:], in_=s01_d[:])
            nc.sync.dma_start(out=e0[:], in_=e_d[0:B0, :])
            nc.sync.dma_start(out=e1[:], in_=e_d[B0:BAND, :])
            nc.sync.dma_start(out=e20[:], in_=e2_d[0:B0, :])
            nc.sync.dma_start(out=e21[:], in_=e2_d[B0:BAND, :])
            nc.sync.dma_start(out=eT[:], in_=eT_d[:])
            nc.sync.dma_start(out=dsb[:], in_=dct_d[:])
            nc.sync.dma_start(out=dTsb[:], in_=dctT_d[:])
            nc.sync.dma_start(out=dloc[:], in_=dloc_d[:])
            nc.sync.dma_start(out=wT[:], in_=wT_d[:])
            nc.vector.memset(dd0[:], 0.0)
            nc.vector.memset(dd1[:], 0.0)

            HL = NLOC // 2      # 1024: half width (bf16 moving max)
            dmae = [nc.sync, nc.gpsimd, nc.sync, nc.scalar]

            for t in range(iters):
                # ---- eigen_y = e.T @ s = e.T @ t1 + (2e).T @ d  (K x NLOC)
                ag_in = dram.tile([K, NLOC], bf, tag="ag_in")
                for h in range(2):
                    hs = slice(h * HL, (h + 1) * HL)
                    ey_ps = ps_big.tile([K, HL], f32, tag="psq")
                    for j in range(2):
                        js = slice(h * HL + j * Q, h * HL + (j + 1) * Q)
                        ps = ey_ps[:, j * Q:(j + 1) * Q]
                        if t == 0:
                            nc.tensor.matmul(ps, e0[:], st0[:, js],
                                             start=True, stop=False)
                            nc.tensor.matmul(ps, e1[:], st1[:, js],
                                             start=False, stop=True)
                        else:
                            nc.tensor.matmul(ps, e20[:], dd0[:, js],
                                             start=True, stop=False)
                            nc.tensor.matmul(ps, e21[:], dd1[:, js],
                                             start=False, stop=False)
                            nc.tensor.matmul(ps, e0[:], t1_0[:, js],
                                             start=False, stop=False)
                            nc.tensor.matmul(ps, e1[:], t1_1[:, js],
                                             start=False, stop=True)
                    if h == 0:
                        nc.scalar.copy(eysb[:, hs], ey_ps[:])
                    else:
                        nc.vector.tensor_copy(eysb[:, hs], ey_ps[:])
                    dmae[h].dma_start(out=ag_in[:, hs], in_=eysb[:, hs])

                ag_out = dram.tile([NCORES, K, NLOC], bf, tag="ag_out")
                nc.gpsimd.collective_compute(
                    "AllGather",
                    mybir.AluOpType.bypass,
                    replica_groups=[list(range(NCORES))],
                    ins=[ag_in.opt()],
                    outs=[ag_out.opt()],
                )
                # gather into (row, k*COL + col) layout, group 0 first so its
                # DCT can begin while group 1's DMAs drain
                for g, ybig in ((0, ybig_a), (1, ybig_b)):
                    for rk in range(NCORES):
                        src = ag_out[rk, g * (K // 2):(g + 1) * (K // 2), :]                             .rearrange("k (r w) -> r k w", r=RLOC)
                        dmae[rk % 4].dma_start(
                            out=ybig[rk * RLOC:(rk + 1) * RLOC, :]
                            .rearrange("r (k w) -> r k w", k=K // 2),
                            in_=src,
                        )


                # ---- DCT denoise: two k-groups of 5, pipelined across
                # engines (PE / DVE / ACT stages of group 0 overlap group 1)
                KB2 = KB // 2          # 640
                KH = K // 2            # 5
                for g in range(2):
                    ybig = ybig_a if g == 0 else ybig_b
                    qt_g = ps_dct.tile([ROW, KB2], f32, tag="dctps")
                    for k in range(KH):
                        nc.tensor.matmul(
                            qt_g[:, k * COL:(k + 1) * COL],
                            ybig[:, k * COL:(k + 1) * COL], dTsb[:],
                            start=True, stop=True)
                    qt_sb = work.tile([ROW, KB2], bf, tag="qt_sb")
                    nc.vector.tensor_copy(qt_sb[:], qt_g[:])

                    ct_g = ps_dct.tile([ROW, KB2], f32, tag="dctps")
                    nc.tensor.matmul(ct_g[:, 0:512], dTsb[:], qt_sb[:, 0:512],
                                     start=True, stop=True)
                    nc.tensor.matmul(ct_g[:, 512:KB2], dTsb[:],
                                     qt_sb[:, 512:KB2],
                                     start=True, stop=True)
                    ct_sb = tmp.tile([ROW, KB2], bf, tag="ct_sb")
                    nc.scalar.copy(ct_sb[:], ct_g[:])
                    cl_sb = tmp.tile([ROW, KB2], bf, tag="cl_sb")
                    nc.vector.tensor_scalar_max(cl_sb[:], ct_sb[:], -TAU)
                    cl2_sb = tmp.tile([ROW, KB2], bf, tag="cl2_sb")
                    nc.vector.tensor_scalar_min(cl2_sb[:], cl_sb[:], TAU)
                    cpt_sb = work.tile([ROW, KB2], bf, tag="cpt_sb")
                    nc.vector.tensor_sub(cpt_sb[:], ct_sb[:], cl2_sb[:])

                    n1_g = ps_big.tile([ROW, KB2], f32, tag="psq")
                    for k in range(KH):
                        nc.tensor.matmul(
                            n1_g[:, k * COL:(k + 1) * COL],
                            cpt_sb[:, k * COL:(k + 1) * COL], dsb[:],
                            start=True, stop=True)
                    n1_sb = work.tile([ROW, KB2], bf, tag="n1_sb")
                    nc.vector.tensor_copy(n1_sb[:], n1_g[:])

                    zl_g = ps_big.tile([RLOC, KB2], f32, tag="psq")
                    nc.tensor.matmul(zl_g[:, 0:512], dloc[:], n1_sb[:, 0:512],
                                     start=True, stop=True)
                    nc.tensor.matmul(zl_g[:, 512:KB2], dloc[:],
                                     n1_sb[:, 512:KB2],
                                     start=True, stop=True)
                    zs_sb = tmp.tile([RLOC, KB2], bf, tag="zs_sb")
                    for k in range(KH):
                        ks = slice(k * COL, (k + 1) * COL)
                        kg = g * KH + k
                        if k % 2 == 0:
                            nc.scalar.copy(zs_sb[:, ks], zl_g[:, ks])
                        else:
                            nc.vector.tensor_copy(zs_sb[:, ks], zl_g[:, ks])
                        dmae[kg % 4].dma_start(
                            out=zsb[kg:kg + 1, :]
                            .rearrange("a (r w) -> a r w", r=RLOC),
                            in_=zs_sb[:, ks],
                        )


                if t == iters - 1:
                    break

                # ---- EZ = e @ z ; state updates pipelined in halves
                # t1 = EZ - d; a = y - t1; d' = clip(a)
                for h in range(2):
                    hs = slice(h * HL, (h + 1) * HL)
                    ez0 = ps_big.tile([B0, HL], f32, tag="psq")
                    ez1 = ps_big.tile([B1, HL], f32, tag="psq")
                    for j in range(2):
                        js = slice(h * HL + j * Q, h * HL + (j + 1) * Q)
                        nc.tensor.matmul(ez0[:, j * Q:(j + 1) * Q],
                                         eT[:, 0:B0], zsb[:, js],
                                         start=True, stop=True)
                        nc.tensor.matmul(ez1[:, j * Q:(j + 1) * Q],
                                         eT[:, B0:BAND], zsb[:, js],
                                         start=True, stop=True)
                    nc.scalar.copy(ez0s[:, hs], ez0[:])
                    nc.scalar.copy(ez1s[:, hs], ez1[:])
                    for (ezs, ddt, yyt, t1t) in (
                        (ez0s, dd0, yw0, t1_0),
                        (ez1s, dd1, yw1, t1_1),
                    ):
                        P = ddt.shape[0]
                        a_ = tmp.tile([P, HL], bf, tag="a_")
                        b_ = tmp.tile([P, HL], bf, tag="b_")
                        nc.vector.tensor_sub(t1t[:, hs], ezs[:, hs], ddt[:, hs])
                        nc.vector.tensor_sub(a_[:], yyt[:, hs], t1t[:, hs])
                        nc.vector.tensor_scalar_max(b_[:], a_[:], -1.0)
                        nc.vector.tensor_scalar_min(ddt[:, hs], b_[:], 1.0)


            # ---- reconstruction: out = W @ z
            osb0 = state.tile([B0, NLOC], f32, tag="osb0")
            osb1 = state.tile([B1, NLOC], f32, tag="osb1")
            for h in range(2):
                hs = slice(h * HL, (h + 1) * HL)
                o0 = ps_big.tile([B0, HL], f32, tag="psq")
                o1 = ps_big.tile([B1, HL], f32, tag="psq")
                for j in range(2):
                    js = slice(h * HL + j * Q, h * HL + (j + 1) * Q)
                    nc.tensor.matmul(o0[:, j * Q:(j + 1) * Q],
                                     wT[:, 0:B0], zsb[:, js],
                                     start=True, stop=True)
                    nc.tensor.matmul(o1[:, j * Q:(j + 1) * Q],
                                     wT[:, B0:BAND], zsb[:, js],
                                     start=True, stop=True)
                nc.scalar.copy(osb0[:, hs], o0[:])
                nc.scalar.copy(osb1[:, hs], o1[:])
            nc.sync.dma_start(out=out_d[0:B0, :], in_=osb0[:])
            nc.sync.dma_start(out=out_d[B0:BAND, :], in_=osb1[:])

    nc.compile()
    return nc


def _get_kernel(iters):
    if iters not in _CACHED:
        _CACHED[iters] = _build_kernel(iters)
    return _CACHED[iters]


def kernel(img, k_subspace, p):
    import ml_dtypes
    bf16 = ml_dtypes.bfloat16
    dtype = np.float32
    img = np.asarray(img, dtype)
    p = dtype(np.asarray(p))
    y_w, s0, e, s = _host_prep(img, p)

    D = _dct_mat(ROW)
    eT = np.ascontiguousarray(e.T)
    wT = np.ascontiguousarray((e * (1.0 / s)[:, None]).T)

    iters = int(globals().get("_ITERS", ITERS))
    nc = _get_kernel(iters)

    def bv(x):
        return np.ascontiguousarray(x).astype(bf16)

    in_maps = []
    for c in range(NCORES):
        cs = slice(c * NLOC, (c + 1) * NLOC)
        in_maps.append({
            "yw0": bv(y_w[0:B0, cs]),
            "yw1": bv(y_w[B0:BAND, cs]),
            "s00": bv(s0[0:B0, cs]),
            "s01": bv(s0[B0:BAND, cs]),
            "e": bv(e),
            "e2": bv(2.0 * e),
            "eT": bv(eT),
            "dct": bv(D),
            "dctT": bv(D.T),
            "dloc": bv(D[:, c * RLOC:(c + 1) * RLOC]),
            "wT": bv(wT),
        })

    from concourse.bass_utils import run_bass_kernel_spmd
    res = run_bass_kernel_spmd(nc, in_maps, list(range(NCORES)),
                               trace=bool(globals().get("_TRACE", False)))
    global _LAST_RESULT
    _LAST_RESULT = res
    y_den = np.concatenate([res.results[c]["out"] for c in range(NCORES)],
                           axis=1)
    return np.ascontiguousarray(y_den.T.reshape(ROW, COL, BAND)).astype(dtype)
